# revision 1
# baseline (speedup 1.0000x reference)
"""Trainium2 Bass kernel for nn_ModalDecoder (embedding_lookup).

Reference computation:
    w  = out_projection_table[idx].reshape(B, F, D, O)      # [B,F,D,O]
    b  = feature_bias_table[idx]                            # [B,F,D]
    xb = x[:, :, None, :] + b[:, None, :, :]                # [B,N,F,D]
    out = einsum('bnfd,bfdo->bnfo', xb, w)                  # [B,N,F,O]

Factorization (avoids the 128MB [B,N,F,D] intermediate):
    out[b, n, f, :] = x[b, n, :] @ W[b, f] + (bias[b, f] @ W[b, f])
The bias term is a per-(b,f) length-O vector, broadcast over n; it is
precomputed on host and added during the PSUM->SBUF drain (per-partition
scalar add on DVE / ACT).

Sharding: 8 cores = 4 values of b x 2 halves of N. Per core:
    y[fo, n] = Wpack[d, fo].T @ xT[d, n] + cvec[fo]
with Wpack = [D, F*O] (host-gathered tables packed side by side), xT the
transposed x half, both bf16 (PSUM accumulates fp32). y is [F*O, N/2] fp16
(upcast on host; the bf16 matmul rounding dominates the error).

Schedule (v6b, HW-measured 22376-22480ns), tuned to measured DMA physics
(per-dma issue ~0.65us of sequencer time; first-transfer ring startup
~0.8us; stream ~390GB/s; a DMA's completion SEMAPHORE becomes visible to
a waiting engine only ~1.5us after its last byte lands, while
engine-to-engine sems take ~40-150ns; ring FIFO orders transfers and
their sem incs):
  - All loads ride the sync HWDGE ring from ONE fused DRAM tensor laid
    out in load order [xtk0|wp0][xtk1|xtk2][xtk3][wp1][wp23][wp45][wp67]
    as 7 transfers with one sem each. The first chunk (256KB) delivers
    everything matmul 1 needs ~3.3us after block start, and each later
    chunk's sem lands before the (cold-rate) PE reaches its gate. Late
    chunks idle the PE, and a >~1us pre-warm idle resets the HAM window
    (~4us lost); merging chunks 2-4 (later sem) measured SLOWER.
  - PE warmup runway: 6x512-free + 7x128-free dummy matmuls keep the PE
    busy from block start (HAM clock-gate warms 1.2->2.4GHz after
    ~2.5-4.5us of sustained activity) and hand over to real matmuls with
    <=107ns granularity.
  - Drains on DVE (749ns/group < 864ns PE group cadence), stores
    alternate rings per group. Tail: group 7's PE work is emitted as two
    256-column halves (s_mm 8 and 9) so h0's drain+store chain launches
    ~0.44us before the final matmuls retire; the halves store as two
    64KB transfers on different rings.
  - ACT and GPSIMD stay compute-idle by necessity: GPSIMD cannot access
    PSUM (BIR verifier), and any ACT activation issued mid-kernel (first
    use triggers an ACT_TABLE_LOAD DMA) wedged the device unrecoverably
    in three separate placements. DVE is the only PSUM-drain engine, so
    its serial g6+g7 drain chain floors the tail. A variant draining g6
    in halves overlapped with same-bank PE writes also hung the HW.
  - Store DMAs carry a dummy sem (HWDGE codegen requires sync info);
    nothing waits on it - the NEFF epilogue's queue DRAIN retires
    in-flight stores.

Per-core HBM traffic: 1.5MB loads + 1MB stores (memory-bound).
"""

import numpy as np
import ml_dtypes

B, N, D, O, F, V = 4, 1024, 512, 64, 16, 64
NH = N // 2            # 512 rows of x per core
FO = F * O             # 1024 packed output columns
KT = D // 128          # 4 contraction chunks
ST = FO // 128         # 8 output-partition chunks
SH = NH // 2           # half-group column split for the tail stores
LD = KT * NH + KT * FO  # 6144 fused load columns

# Fused load-buffer column offsets (load order).
XT_COL = {0: 0, 1: 1024, 2: 1536, 3: 2048}       # xt k-chunk -> col
WP_COL = {0: 512, 1: 2560, 2: 3072, 3: 3584,
          4: 4096, 5: 4608, 6: 5120, 7: 5632}    # wp s-group -> col
# Load transfers: (col_start, col_end); chunk j gets its own semaphore.
LD_CHUNKS = [(0, 1024), (1024, 2048), (2048, 2560), (2560, 3072),
             (3072, 4096), (4096, 5120), (5120, 6144)]
HH = 256               # group-7 PE split: h0 = 0:HH, h1 = HH:NH

_cache: dict = {}


def _build_program(with_clears=True):
    # with_clears=True is the real (HW) program. The False variant is for
    # CoreSim validation: it enables the race detector, memsets the warmup
    # scratch (CoreSim rejects reads of uninitialized SBUF; on HW the
    # warmup inputs are garbage by design and never observed), and adds
    # completion sems to the store DMAs (race-detector requirement; the HW
    # variant relies on the epilogue DRAIN instead).
    import concourse.bass as bass
    import concourse.mybir as mybir

    bf16 = mybir.dt.bfloat16
    f16 = mybir.dt.float16
    f32 = mybir.dt.float32

    nc = bass.Bass(
        "TRN2",
        target_bir_lowering=False,
        debug=False,
        num_devices=8,
        detect_race_conditions=not with_clears,
    )

    ld_d = nc.dram_tensor("ld", [128, LD], bf16, kind="ExternalInput")
    cv_d = nc.dram_tensor("cv", [128, ST], f32, kind="ExternalInput")
    y_d = nc.dram_tensor("y", [FO, NH], f16, kind="ExternalOutput")

    yv = y_d.ap().rearrange("(g p) n -> p g n", p=128)  # [128, ST, NH]

    with (
        nc.sbuf_tensor("ld_sb", [128, LD], bf16) as ld_sb,
        nc.sbuf_tensor("cv_sb", [128, ST], f32) as cv_sb,
        nc.sbuf_tensor("out_sb", [128, ST, NH], f16) as out_sb,
        nc.sbuf_tensor("scr_sb", [128, NH], bf16) as scr_sb,
        nc.psum_tensor([128, ST, NH], f32) as ps,
        nc.semaphore("s_l1") as s_l1,
        nc.semaphore("s_l2") as s_l2,
        nc.semaphore("s_l3") as s_l3,
        nc.semaphore("s_l4") as s_l4,
        nc.semaphore("s_l5") as s_l5,
        nc.semaphore("s_l6") as s_l6,
        nc.semaphore("s_l7") as s_l7,
        nc.semaphore("s_cv") as s_cv,
        nc.semaphore("s_ws") as s_ws,
        nc.semaphore("s_mm") as s_mm,
        nc.semaphore("s_dve_sync") as s_dve_sync,
        nc.semaphore("s_dve_act") as s_dve_act,
        nc.semaphore("s_st") as s_st,
        nc.Block() as block,
    ):
        ld_sems = [s_l1, s_l2, s_l3, s_l4, s_l5, s_l6, s_l7]

        def store(eng, dst, src):
            # HWDGE DMAs must carry a sem update (codegen requirement);
            # nothing waits on s_st -- the epilogue DRAIN handles retirement.
            eng.dma_start(dst, src).then_inc(s_st, 16)

        @block.sync
        def _(sync):
            # All loads on one ring, in PE-gating order; chunk j's sem
            # implies chunks <j landed (per-engine FIFO).
            for (c0, c1), sem in zip(LD_CHUNKS, ld_sems):
                sync.dma_start(ld_sb[:, c0:c1], ld_d.ap()[:, c0:c1]).then_inc(
                    sem, 16
                )
            # Even-group stores, then the first half of group 7.
            for j, s in enumerate((0, 2, 4, 6)):
                sync.wait_ge(s_dve_sync, j + 1)
                store(sync, yv[:, s, :], out_sb[:, s, :])
            sync.wait_ge(s_dve_sync, 5)      # g7h0 drained
            store(sync, yv[:, 7, 0:HH], out_sb[:, 7, 0:HH])
            # No final completion wait: the framework epilogue's DRAIN
            # retires in-flight DMAs.

        @block.scalar
        def _(scalar):
            # cv primes this ring's DMA path; only DVE consumes it.
            # (No ACT compute anywhere: an activation instruction's
            # ACT_TABLE_LOAD DMA wedged the HW when issued concurrently
            # with this kernel's dynamic-DMA traffic.)
            scalar.dma_start(cv_sb[:], cv_d.ap()).then_inc(s_cv, 16)
            for j, s in enumerate((1, 3, 5)):
                scalar.wait_ge(s_dve_act, j + 1)
                store(scalar, yv[:, s, :], out_sb[:, s, :])
            # Tail: second half of group 7.
            scalar.wait_ge(s_dve_act, 4)
            store(scalar, yv[:, 7, HH:NH], out_sb[:, 7, HH:NH])

        @block.tensor
        def _(tensor):
            # Warmup runway: keeps the PE busy from block start until the
            # first load gate. Coarse then fine, so the handover to real
            # work wastes <=107ns. scr_sb is never written on HW.
            if not with_clears:
                tensor.wait_ge(s_ws, 1)
            for _ in range(6):
                nc.tensor.matmul(
                    ps[:, ST - 1, :],
                    scr_sb[:, :128],
                    scr_sb[:],
                    start=True,
                    stop=True,
                )
            for _ in range(7):
                nc.tensor.matmul(
                    ps[:, ST - 1, 0:128],
                    scr_sb[:, :128],
                    scr_sb[:, 0:128],
                    start=True,
                    stop=True,
                )
            # Group-serial accumulation: group s finishes after its own 4
            # matmuls, so DVE drains + stores pipeline behind PE.
            for s in range(ST):
                if s == 0:
                    tensor.wait_ge(s_l1, 16)    # xt k0 + wp g0
                elif s == 1:
                    tensor.wait_ge(s_l4, 16)    # wp g1
                elif s == 2:
                    tensor.wait_ge(s_l5, 16)    # wp g2,g3
                elif s == 4:
                    tensor.wait_ge(s_l6, 16)    # wp g4,g5
                elif s == 6:
                    tensor.wait_ge(s_l7, 16)    # wp g6,g7
                if s == ST - 1:
                    # Last group in column halves so the tail drain+store
                    # chain starts before the final matmuls retire.
                    for c0, c1 in ((0, HH), (HH, NH)):
                        for k in range(KT):
                            inst = nc.tensor.matmul(
                                ps[:, s, c0:c1],
                                ld_sb[:, WP_COL[s] + k * 128:
                                      WP_COL[s] + (k + 1) * 128],
                                ld_sb[:, XT_COL[k] + c0:XT_COL[k] + c1],
                                start=(k == 0),
                                stop=(k == KT - 1),
                            )
                            if k == KT - 1:
                                inst.then_inc(s_mm, 1)
                    continue
                for k in range(KT):
                    if s == 0 and k == 1:
                        tensor.wait_ge(s_l2, 16)    # xt k1,k2
                    elif s == 0 and k == 3:
                        tensor.wait_ge(s_l3, 16)    # xt k3
                    inst = nc.tensor.matmul(
                        ps[:, s, :],
                        ld_sb[:, WP_COL[s] + k * 128:WP_COL[s] + (k + 1) * 128],
                        ld_sb[:, XT_COL[k]:XT_COL[k] + NH],
                        start=(k == 0),
                        stop=(k == KT - 1),
                    )
                    if k == KT - 1:
                        inst.then_inc(s_mm, 1)

        @block.vector
        def _(vector):
            if not with_clears:
                vector.memset(scr_sb[:], 0).then_inc(s_ws, 1)
            vector.wait_ge(s_cv, 16)  # cv loaded
            for s in range(ST - 1):
                vector.wait_ge(s_mm, s + 1)
                inst = nc.vector.tensor_scalar_add(
                    out_sb[:, s, :], ps[:, s, :], cv_sb[:, s:s + 1]
                )
                if s % 2 == 0:
                    inst.then_inc(s_dve_sync, 1)
                else:
                    inst.then_inc(s_dve_act, 1)
            # Group-7 halves (s_mm: g7h0=8, g7h1=9).
            vector.wait_ge(s_mm, ST)
            nc.vector.tensor_scalar_add(
                out_sb[:, 7, 0:HH], ps[:, 7, 0:HH], cv_sb[:, 7:8]
            ).then_inc(s_dve_sync, 1)
            vector.wait_ge(s_mm, ST + 1)
            nc.vector.tensor_scalar_add(
                out_sb[:, 7, HH:NH], ps[:, 7, HH:NH], cv_sb[:, 7:8]
            ).then_inc(s_dve_act, 1)

    return nc


def _get_program():
    nc = _cache.get("nc")
    if nc is None:
        nc = _build_program()
        _cache["nc"] = nc
    return nc


def _prep_in_maps(x, idx, fbt, opt):
    bf = ml_dtypes.bfloat16
    in_maps = []
    for b in range(B):
        w = opt[idx[b]].reshape(F, D, O)                     # [F,D,O] f32
        wpack = w.transpose(1, 0, 2).reshape(KT, 128, ST, 128)  # [k,p,s,c]
        wp_host = np.ascontiguousarray(
            wpack.transpose(1, 2, 0, 3).reshape(128, KT * FO)
        ).astype(bf)                                         # [p, s*512+k*128+c]
        bias = fbt[idx[b]]                                   # [F,D]
        cvec = np.einsum("fd,fdo->fo", bias, w).reshape(FO).astype(np.float32)
        cv = np.ascontiguousarray(cvec.reshape(ST, 128).T)   # [128, ST]
        for h in range(2):
            xtT = x[b, h * NH:(h + 1) * NH, :].T             # [D, NH]
            xt_host = np.ascontiguousarray(
                xtT.reshape(KT, 128, NH).transpose(1, 0, 2).reshape(128, KT * NH)
            ).astype(bf)                                     # [128, k*NH+col]
            ldh = np.empty((128, LD), dtype=bf)
            for k in range(KT):
                ldh[:, XT_COL[k]:XT_COL[k] + NH] = xt_host[
                    :, k * NH:(k + 1) * NH
                ]
            for s in range(ST):
                ldh[:, WP_COL[s]:WP_COL[s] + 512] = wp_host[
                    :, s * 512:(s + 1) * 512
                ]
            in_maps.append({"ld": ldh, "cv": cv})
    return in_maps


def _assemble(results):
    out = np.empty((B, N, F, O), dtype=np.float32)
    for c in range(8):
        b, h = divmod(c, 2)
        y = np.asarray(results[c]["y"]).astype(np.float32)   # [FO, NH]
        out[b, h * NH:(h + 1) * NH] = y.reshape(F, O, NH).transpose(2, 0, 1)
    return out


def _run(x, idx, feature_bias_table, out_projection_table, **run_kwargs):
    from concourse.bass_utils import run_bass_kernel_spmd

    x = np.asarray(x, dtype=np.float32)
    idx = np.asarray(idx).astype(np.int64)
    fbt = np.asarray(feature_bias_table, dtype=np.float32)
    opt = np.asarray(out_projection_table, dtype=np.float32)

    nc = _get_program()
    in_maps = _prep_in_maps(x, idx, fbt, opt)
    res = run_bass_kernel_spmd(nc, in_maps, core_ids=list(range(8)), **run_kwargs)
    return _assemble(res.results), res


def kernel(x, idx, feature_bias_table, out_projection_table):
    out, _ = _run(x, idx, feature_bias_table, out_projection_table)
    return out

